# revision 10
# baseline (speedup 1.0000x reference)
"""Distributed kNN retrieval + subjective-logic fusion kernel for 8 Trainium2 cores.

Strategy (classic distributed kNN per the sharding hint):
  - Shard the memory bank across 8 cores along N (12500 rows each, zero-padded
    to 12544).  Host prepares normalized, transposed fp8e4 operand layouts
    (layout/dtype prep only; all O(B*N*D) compute runs on device).
  - Each core computes cosine sims for all 1024 queries against its shard
    using fp8 DoubleRow matmuls (both 128-deep k-planes of the 256-dim
    contraction in one instruction, fp32 PSUM accumulate), in units of 2048
    sims, and reduces each unit to 1024 pair-maxes
    hm[i] = max(s[u*2048+i], s[u*2048+1024+i]) (halving the candidate
    plane; candidates become index pairs):
      * mode S units: ACT copies the whole unit PSUM->bf16, DVE pair-maxes
        the bf16 copy in the DVE 2x mode (0.5 cycles/elem)
      * mode M units: ACT copies only the first half; DVE pair-maxes the
        bf16 copy against the second half straight out of PSUM (hw allows
        one PSUM operand per vector op)
    The 6272-entry bf16 pair-max plane per query is DMA'd out; that is the
    kernel's entire device->host traffic (no on-device top-k needed).
  - Host runs an exact two-level top-16 over each core's pair-max plane
    (grouped max + argpartition, standard top-k-of-group-maxes covering
    argument), resolves the 16 winning pairs to 2x16 candidate indices,
    rescores the 8x32 candidates per query with exact fp32 dot products,
    then applies softmax and the Dirichlet/DST opinion fusion (exactly
    mirrors the fp32 reference).
"""
import sys
sys.path.insert(0, '/opt/trn_rl_repo')
from contextlib import ExitStack

import numpy as np
import ml_dtypes

import concourse.bass as bass
import concourse.tile as tile
from concourse import mybir, bacc, bass_utils

EPS = 1e-8
TEMPERATURE = 0.07

B, D, N, K = 1024, 256, 100000, 2
NCORES = 8
NLOC_REAL = N // NCORES          # 12500
UNIT = 2048                      # sims per full pair-unit (4 fp32 PSUM banks)
NU = 6                           # full units
TAIL = 256                       # tail unit sims (pairs split 128/128)
NLOC = NU * UNIT + TAIL          # 12544 padded shard size
HM = NLOC // 2                   # 6272 pair-maxes per query
QT = 128                         # queries per tile
NQT = B // QT                    # 8 query tiles
TOPK = 16

# unit modes: 'S' = ACT copies both halves, DVE 2x pair-max on bf16;
# 'M' = ACT copies first half, DVE pair-max bf16 vs PSUM (engine balance knob)
MODES = ("S", "M", "M", "M", "M", "M")

_cache = {}


def _build_program():
    nc = bacc.Bacc("TRN2", target_bir_lowering=False, debug=False)

    mt = nc.dram_tensor("mt", [128, 2, NLOC], mybir.dt.float8e4, kind="ExternalInput")
    qt = nc.dram_tensor("qt", [128, 2, B], mybir.dt.float8e4, kind="ExternalInput")
    ov = nc.dram_tensor("ov", [B, HM], mybir.dt.bfloat16, kind="ExternalOutput")

    with tile.TileContext(nc) as tc, ExitStack() as ctx:
        const = ctx.enter_context(tc.tile_pool(name="const", bufs=1))
        tmp = ctx.enter_context(tc.tile_pool(name="tmp", bufs=3))
        psum = ctx.enter_context(tc.tile_pool(name="psum", bufs=2, space="PSUM"))

        qt_sb = const.tile([128, 2, B], mybir.dt.float8e4)
        nc.gpsimd.dma_start(qt_sb[:], qt.ap())
        # chunked memory load so the first matmuls start early; spread across
        # queues with exec depth > 0 (SP's depth-0 queue blocks per transfer)
        mt_sb = const.tile([128, 2, NLOC], mybir.dt.float8e4)
        mt_edges = [0, 512, 1024, 2048] + list(range(4096, NLOC, 2048)) + [NLOC]
        load_engines = [nc.sync, nc.gpsimd]
        for i, (a, b) in enumerate(zip(mt_edges[:-1], mt_edges[1:])):
            load_engines[i % len(load_engines)].dma_start(
                mt_sb[:, :, a:b], mt.ap()[:, :, a:b])

        # pair-max planes, ring over q-tiles
        NHM = 3
        hms = [const.tile([128, HM], mybir.dt.bfloat16, tag=f"hm{i}",
                          name=f"hm{i}") for i in range(NHM)]

        def pair_unit(t, base, cols, mode, hm):
            # matmul `cols` sims into PSUM, reduce to cols//2 pair-maxes
            half = cols // 2
            ps = psum.tile([128, UNIT], mybir.dt.float32)
            for s in range(0, cols, 512):
                w = min(512, cols - s)
                nc.tensor.matmul(
                    ps[:, s:s + w],
                    qt_sb[:, :, t * QT:(t + 1) * QT],
                    mt_sb[:, :, base + s:base + s + w],
                    start=True, stop=True,
                    perf_mode=mybir.MatmulPerfMode.DoubleRow,
                )
            dst = hm[:, base // 2:base // 2 + half]
            tp = tmp.tile([128, UNIT], mybir.dt.bfloat16, tag="tp")
            if mode == "S":
                nc.scalar.copy(tp[:, 0:cols], ps[:, 0:cols])
                nc.vector.tensor_tensor(dst, tp[:, 0:half], tp[:, half:cols],
                                        mybir.AluOpType.max)
            else:
                nc.scalar.copy(tp[:, 0:half], ps[:, 0:half])
                nc.vector.tensor_tensor(dst, tp[:, 0:half], ps[:, half:cols],
                                        mybir.AluOpType.max)

        for t in range(NQT):
            hm = hms[t % NHM]
            for u in range(NU):
                pair_unit(t, u * UNIT, UNIT, MODES[u], hm)
            pair_unit(t, NU * UNIT, TAIL, "M", hm)
            # ship the tile's pair-max plane; host does the top-k.
            # gpsimd's SWDGE queue has exec depth 4 so the transfer does not
            # hold a sequencer (SP's depth-0 HWDGE queue would serialize).
            nc.gpsimd.dma_start(ov.ap()[t * QT:(t + 1) * QT, :], hm[:])

    nc.compile()
    return nc


def _get_program():
    if "nc" not in _cache:
        _cache["nc"] = _build_program()
    return _cache["nc"]


def _prep_inputs(query, memory_feat):
    qn = np.sqrt((query.astype(np.float32) ** 2).sum(-1, keepdims=True))
    qhat = query / np.clip(qn, EPS, None)
    mn = np.sqrt((memory_feat.astype(np.float32) ** 2).sum(-1, keepdims=True))
    mhat = memory_feat / np.clip(mn, EPS, None)

    # qt: (128, 2, B) fp8 with qt[p, h, b] = qhat[b, h*128+p]
    qtl = np.ascontiguousarray(
        qhat.T.reshape(2, 128, B).transpose(1, 0, 2)
    ).astype(ml_dtypes.float8_e4m3)

    # memory shards: (128, 2, NLOC) fp8 with mt[p, h, j] = mhat[c*12500+j, h*128+p]
    mts = []
    for c in range(NCORES):
        slab = mhat[c * NLOC_REAL:(c + 1) * NLOC_REAL]
        slab = np.concatenate(
            [slab, np.zeros((NLOC - NLOC_REAL, D), np.float32)], axis=0
        )
        mtl = np.ascontiguousarray(
            slab.T.reshape(2, 128, NLOC).transpose(1, 0, 2)
        ).astype(ml_dtypes.float8_e4m3)
        mts.append(mtl)
    return qhat, mhat, qtl, mts


def _fuse_host(topv, topi, memory_evidence, model_evidence):
    """Exact fp32 mirror of the reference softmax + DST fusion."""
    f32 = np.float32
    w = topv.astype(f32) / f32(TEMPERATURE)
    w = w - w.max(-1, keepdims=True)
    w = np.exp(w)
    w = w / w.sum(-1, keepdims=True)

    ev = memory_evidence[topi]                      # (B, k, K)
    alpha_r = f32(1.0) + np.einsum("bk,bkc->bc", w, ev.astype(f32))
    alpha_m = model_evidence.astype(f32) + f32(1.0)

    def alpha_to_belief_u(alpha):
        Kd = alpha.shape[-1]
        S = np.clip(alpha.sum(-1, keepdims=True), EPS, None)
        b = np.clip((alpha - 1.0) / S, 0.0, None)
        u = np.clip(Kd / S, EPS, 1.0 - EPS)
        b_sum = b.sum(-1, keepdims=True)
        target = np.clip(1.0 - u, EPS, None)
        b = b * (target / np.clip(b_sum, EPS, None))
        return b.astype(f32), u.astype(f32)

    def combine_two_opinions(b1, u1, b2, u2):
        total_pair = b1.sum(-1, keepdims=True) * b2.sum(-1, keepdims=True)
        dot_same = (b1 * b2).sum(-1, keepdims=True)
        C = total_pair - dot_same
        S = np.clip(1.0 - C, EPS, None)
        b = (b1 * b2 + b1 * u2 + b2 * u1) / S
        u = u1 * u2 / S
        b = np.clip(b, 0.0, None)
        u = np.clip(u, EPS, 1.0 - EPS)
        b_sum = b.sum(-1, keepdims=True)
        b = b * ((1.0 - u) / np.clip(b_sum, EPS, None))
        return b.astype(f32), u.astype(f32)

    def opinion_to_alpha(b, u):
        Kd = b.shape[-1]
        u = np.clip(u, EPS, 1.0 - EPS)
        S = Kd / u
        alpha = b * S + 1.0
        return np.clip(alpha, 1.0 + EPS, None).astype(f32)

    b_m, u_m = alpha_to_belief_u(alpha_m)
    b_r, u_r = alpha_to_belief_u(alpha_r)
    b_f, u_f = combine_two_opinions(b_m, u_m, b_r, u_r)
    return opinion_to_alpha(b_f, u_f)


def _pair_indices(h):
    """Map hm plane index -> (first, second) item positions in the slab."""
    full = h < NU * UNIT // 2
    u = h // (UNIT // 2)
    i = h % (UNIT // 2)
    i0_full = u * UNIT + i
    it = h - NU * UNIT // 2
    i0_tail = NU * UNIT + it
    i0 = np.where(full, i0_full, i0_tail)
    i1 = np.where(full, i0_full + UNIT // 2, i0_tail + TAIL // 2)
    return i0, i1


def _bf16_to_f32(x_bf16):
    """Fast bf16 -> fp32 (bit shift; avoids slow ml_dtypes casting)."""
    u = x_bf16.view(np.uint16).astype(np.uint32) << 16
    return u.view(np.float32)


def _top16_rows(regs):
    """Exact top-16 positions per row of (B, HM) via two-level group maxes."""
    GH = 16                                          # host group size
    ng = regs.shape[1] // GH                         # 392 groups
    g = regs.reshape(B, ng, GH)
    gmax = g.max(axis=2)                             # (B, ng)
    gsel = np.argpartition(-gmax, TOPK - 1, axis=1)[:, :TOPK]   # (B, 16)
    cand = np.take_along_axis(g, gsel[:, :, None], axis=1)      # (B, 16, GH)
    cand = cand.reshape(B, TOPK * GH)
    fsel = np.argpartition(-cand, TOPK - 1, axis=1)[:, :TOPK]   # (B, 16)
    grp = np.take_along_axis(gsel, fsel // GH, axis=1)
    return grp * GH + fsel % GH                      # hm indices (B, 16)


def kernel(query, memory_feat, memory_evidence, model_evidence, top_k):
    top_k = int(top_k)
    assert top_k == TOPK

    query = np.asarray(query, dtype=np.float32)
    memory_feat = np.asarray(memory_feat, dtype=np.float32)
    memory_evidence = np.asarray(memory_evidence, dtype=np.float32)
    model_evidence = np.asarray(model_evidence, dtype=np.float32)

    nc = _get_program()
    qhat, mhat, qtl, mts = _prep_inputs(query, memory_feat)

    in_maps = [{"mt": mts[c], "qt": qtl} for c in range(NCORES)]
    res = bass_utils.run_bass_kernel_spmd(nc, in_maps, core_ids=list(range(NCORES)))
    _cache["last_results"] = res

    # host-side top-16 pairs per core, resolved to 2x16 candidate indices
    cand_idx = np.empty((B, NCORES * 2 * TOPK), dtype=np.int64)
    for c in range(NCORES):
        regs = _bf16_to_f32(res.results[c]["ov"])    # (B, HM) pair values
        h = _top16_rows(regs)                        # (B, 16) hm indices
        i0, i1 = _pair_indices(h)
        idx = np.concatenate([i0, i1], axis=1)       # (B, 32) slab positions
        valid = idx < NLOC_REAL
        gidx = c * NLOC_REAL + np.clip(idx, 0, NLOC_REAL - 1)
        gidx[~valid] = -1
        cand_idx[:, c * 2 * TOPK:(c + 1) * 2 * TOPK] = gidx

    # exact fp32 rescore of the 256 candidates per query
    safe_idx = np.clip(cand_idx, 0, N - 1)
    mh_c = mhat[safe_idx]                            # (B, 256, D)
    s = np.einsum("bd,bkd->bk", qhat, mh_c).astype(np.float32)
    s[cand_idx < 0] = -np.inf

    order = np.argsort(-s, axis=1, kind="stable")[:, :TOPK]
    topv = np.take_along_axis(s, order, axis=1)
    topi = np.take_along_axis(cand_idx, order, axis=1)

    return _fuse_host(topv, topi, memory_evidence, model_evidence)


# revision 23
# speedup vs baseline: 1.3289x; 1.3289x over previous
"""Distributed kNN retrieval + subjective-logic fusion kernel for 8 Trainium2 cores.

Strategy (classic distributed kNN per the sharding hint):
  - Shard the memory bank across 8 cores along N (12500 rows each, zero-padded
    to 12544).  Host prepares normalized, transposed fp8e4 operand layouts
    (layout/dtype prep only; all O(B*N*D) compute runs on device).
  - Each core computes cosine sims for all 1024 queries against its shard
    using fp8 DoubleRow matmuls (both 128-deep k-planes of the 256-dim
    contraction in one instruction, fp32 PSUM accumulate), in units of 2048
    sims, and reduces each unit to 1024 pair-maxes
    hm[i] = max(s[u*2048+i], s[u*2048+1024+i]) (halving the candidate
    plane; candidates become index pairs):
      * mode S units: ACT copies the whole unit PSUM->bf16, DVE pair-maxes
        the bf16 copy in the DVE 2x mode (0.5 cycles/elem)
      * mode M units: ACT copies only the first half; DVE pair-maxes the
        bf16 copy against the second half straight out of PSUM (hw allows
        one PSUM operand per vector op)
    The 6272-entry bf16 pair-max plane per query is DMA'd out; that is the
    kernel's entire device->host traffic (no on-device top-k needed).
  - Host runs an exact two-level top-16 over each core's pair-max plane
    (grouped max + argpartition, standard top-k-of-group-maxes covering
    argument), resolves the 16 winning pairs to 2x16 candidate indices,
    rescores the 8x32 candidates per query with exact fp32 dot products,
    then applies softmax and the Dirichlet/DST opinion fusion (exactly
    mirrors the fp32 reference).
"""
import sys
sys.path.insert(0, '/opt/trn_rl_repo')
from contextlib import ExitStack

import numpy as np
import ml_dtypes

import concourse.bass as bass
import concourse.tile as tile
from concourse import mybir, bacc, bass_utils

EPS = 1e-8
TEMPERATURE = 0.07

B, D, N, K = 1024, 256, 100000, 2
NCORES = 8
NLOC_REAL = N // NCORES          # 12500
UNIT = 2048                      # sims per full pair-unit (4 fp32 PSUM banks)
NU = 6                           # full units
TAIL = 256                       # tail unit sims (pairs split 128/128)
NLOC = NU * UNIT + TAIL          # 12544 padded shard size
HM = NLOC // 2                   # 6272 pair-maxes per query
QT = 128                         # queries per tile
NQT = B // QT                    # 8 query tiles
TOPK = 16

# unit modes: 'S' = ACT copies both halves, DVE 2x pair-max on bf16;
# 'M' = ACT copies first half, DVE pair-max bf16 vs PSUM;
# 'H' = hybrid: first HF pairs S-style, rest M-style (engine balance knob)
MODES = ("M", "M", "H", "M", "M", "M")
HF = 960

_cache = {}


def _build_program():
    nc = bacc.Bacc("TRN2", target_bir_lowering=False, debug=False)

    mt = nc.dram_tensor("mt", [128, 2, NLOC], mybir.dt.float8e4, kind="ExternalInput")
    qt = nc.dram_tensor("qt", [128, 2, B], mybir.dt.float8e4, kind="ExternalInput")
    ov = nc.dram_tensor("ov", [B, HM], mybir.dt.bfloat16, kind="ExternalOutput")

    with tile.TileContext(nc) as tc, ExitStack() as ctx:
        const = ctx.enter_context(tc.tile_pool(name="const", bufs=1))
        tmp = ctx.enter_context(tc.tile_pool(name="tmp", bufs=3))
        psum = ctx.enter_context(tc.tile_pool(name="psum", bufs=2, space="PSUM"))

        qt_sb = const.tile([128, 2, B], mybir.dt.float8e4)
        # tile 0's stationary first so its matmuls are not gated on the rest
        nc.sync.dma_start(qt_sb[:, :, 0:QT], qt.ap()[:, :, 0:QT])
        nc.gpsimd.dma_start(qt_sb[:, :, QT:B], qt.ap()[:, :, QT:B])
        # chunked memory load so the first matmuls start early; spread across
        # queues with exec depth > 0 (SP's depth-0 queue blocks per transfer)
        mt_sb = const.tile([128, 2, NLOC], mybir.dt.float8e4)
        mt_edges = [0, 512, 1024, 2048] + list(range(4096, NLOC, 2048)) + [NLOC]
        spans = list(zip(mt_edges[:-1], mt_edges[1:]))
        spans = [spans[-1]] + spans[:-1]   # tail chunk first (tiles start there)
        load_engines = [nc.sync, nc.gpsimd]
        for i, (a, b) in enumerate(spans):
            load_engines[i % len(load_engines)].dma_start(
                mt_sb[:, :, a:b], mt.ap()[:, :, a:b])

        # pair-max planes, ring over q-tiles
        NHM = 4
        hms = [const.tile([128, HM], mybir.dt.bfloat16, tag=f"hm{i}",
                          name=f"hm{i}") for i in range(NHM)]

        def pair_unit(t, base, cols, mode, hm):
            # matmul `cols` sims into two 2-bank PSUM tiles (A = first half,
            # B = second half); reduce to cols//2 pair-maxes.  ACT's copy of
            # A frees that tile early so the PE stays ahead of the reducers.
            half = cols // 2
            psa = psum.tile([128, UNIT // 2], mybir.dt.float32, tag="psa")
            psb = psum.tile([128, UNIT // 2], mybir.dt.float32, tag="psb")
            for ps, off in ((psa, 0), (psb, half)):
                for s in range(0, half, 512):
                    w = min(512, half - s)
                    nc.tensor.matmul(
                        ps[:, s:s + w],
                        qt_sb[:, :, t * QT:(t + 1) * QT],
                        mt_sb[:, :, base + off + s:base + off + s + w],
                        start=True, stop=True,
                        perf_mode=mybir.MatmulPerfMode.DoubleRow,
                    )
            dst = hm[:, base // 2:base // 2 + half]
            tp = tmp.tile([128, UNIT], mybir.dt.bfloat16, tag="tp")
            nc.scalar.copy(tp[:, 0:half], psa[:, 0:half])
            if mode == "S":
                nc.scalar.copy(tp[:, half:cols], psb[:, 0:half])
                nc.vector.tensor_tensor(dst, tp[:, 0:half], tp[:, half:cols],
                                        mybir.AluOpType.max)
            elif mode == "H":
                nc.scalar.copy(tp[:, half:half + HF], psb[:, 0:HF])
                nc.vector.tensor_tensor(dst[:, 0:HF], tp[:, 0:HF],
                                        tp[:, half:half + HF],
                                        mybir.AluOpType.max)
                nc.vector.tensor_tensor(dst[:, HF:half], tp[:, HF:half],
                                        psb[:, HF:half], mybir.AluOpType.max)
            else:
                nc.vector.tensor_tensor(dst, tp[:, 0:half], psb[:, 0:half],
                                        mybir.AluOpType.max)

        # ship slices as soon as their units are done; gpsimd's SWDGE queue
        # has exec depth 4 so transfers do not hold a sequencer (SP's depth-0
        # HWDGE queue would serialize).  The tail unit runs first so the
        # tile's last DMA is a plain 2048-slice, not a dependent straggler.
        ship_after = {1: (0, 2048), 3: (2048, 4096), 4: (4096, 5120),
                      5: (5120, 6144)}
        for t in range(NQT):
            hm = hms[t % NHM]
            for u in range(NU):
                pair_unit(t, u * UNIT, UNIT, MODES[u], hm)
                if u in ship_after:
                    a, b = ship_after[u]
                    nc.gpsimd.dma_start(ov.ap()[t * QT:(t + 1) * QT, a:b],
                                        hm[:, a:b])
            pair_unit(t, NU * UNIT, TAIL, "M", hm)
            nc.gpsimd.dma_start(ov.ap()[t * QT:(t + 1) * QT, 6144:HM],
                                hm[:, 6144:HM])

    nc.compile()
    return nc


def _get_program():
    if "nc" not in _cache:
        _cache["nc"] = _build_program()
    return _cache["nc"]


def _prep_inputs(query, memory_feat):
    qn = np.sqrt((query.astype(np.float32) ** 2).sum(-1, keepdims=True))
    qhat = query / np.clip(qn, EPS, None)
    mn = np.sqrt((memory_feat.astype(np.float32) ** 2).sum(-1, keepdims=True))
    mhat = memory_feat / np.clip(mn, EPS, None)

    # qt: (128, 2, B) fp8 with qt[p, h, b] = qhat[b, h*128+p]
    qtl = np.ascontiguousarray(
        qhat.T.reshape(2, 128, B).transpose(1, 0, 2)
    ).astype(ml_dtypes.float8_e4m3)

    # memory shards: (128, 2, NLOC) fp8 with mt[p, h, j] = mhat[c*12500+j, h*128+p]
    mts = []
    for c in range(NCORES):
        slab = mhat[c * NLOC_REAL:(c + 1) * NLOC_REAL]
        slab = np.concatenate(
            [slab, np.zeros((NLOC - NLOC_REAL, D), np.float32)], axis=0
        )
        mtl = np.ascontiguousarray(
            slab.T.reshape(2, 128, NLOC).transpose(1, 0, 2)
        ).astype(ml_dtypes.float8_e4m3)
        mts.append(mtl)
    return qhat, mhat, qtl, mts


def _fuse_host(topv, topi, memory_evidence, model_evidence):
    """Exact fp32 mirror of the reference softmax + DST fusion."""
    f32 = np.float32
    w = topv.astype(f32) / f32(TEMPERATURE)
    w = w - w.max(-1, keepdims=True)
    w = np.exp(w)
    w = w / w.sum(-1, keepdims=True)

    ev = memory_evidence[topi]                      # (B, k, K)
    alpha_r = f32(1.0) + np.einsum("bk,bkc->bc", w, ev.astype(f32))
    alpha_m = model_evidence.astype(f32) + f32(1.0)

    def alpha_to_belief_u(alpha):
        Kd = alpha.shape[-1]
        S = np.clip(alpha.sum(-1, keepdims=True), EPS, None)
        b = np.clip((alpha - 1.0) / S, 0.0, None)
        u = np.clip(Kd / S, EPS, 1.0 - EPS)
        b_sum = b.sum(-1, keepdims=True)
        target = np.clip(1.0 - u, EPS, None)
        b = b * (target / np.clip(b_sum, EPS, None))
        return b.astype(f32), u.astype(f32)

    def combine_two_opinions(b1, u1, b2, u2):
        total_pair = b1.sum(-1, keepdims=True) * b2.sum(-1, keepdims=True)
        dot_same = (b1 * b2).sum(-1, keepdims=True)
        C = total_pair - dot_same
        S = np.clip(1.0 - C, EPS, None)
        b = (b1 * b2 + b1 * u2 + b2 * u1) / S
        u = u1 * u2 / S
        b = np.clip(b, 0.0, None)
        u = np.clip(u, EPS, 1.0 - EPS)
        b_sum = b.sum(-1, keepdims=True)
        b = b * ((1.0 - u) / np.clip(b_sum, EPS, None))
        return b.astype(f32), u.astype(f32)

    def opinion_to_alpha(b, u):
        Kd = b.shape[-1]
        u = np.clip(u, EPS, 1.0 - EPS)
        S = Kd / u
        alpha = b * S + 1.0
        return np.clip(alpha, 1.0 + EPS, None).astype(f32)

    b_m, u_m = alpha_to_belief_u(alpha_m)
    b_r, u_r = alpha_to_belief_u(alpha_r)
    b_f, u_f = combine_two_opinions(b_m, u_m, b_r, u_r)
    return opinion_to_alpha(b_f, u_f)


def _pair_indices(h):
    """Map hm plane index -> (first, second) item positions in the slab."""
    full = h < NU * UNIT // 2
    u = h // (UNIT // 2)
    i = h % (UNIT // 2)
    i0_full = u * UNIT + i
    it = h - NU * UNIT // 2
    i0_tail = NU * UNIT + it
    i0 = np.where(full, i0_full, i0_tail)
    i1 = np.where(full, i0_full + UNIT // 2, i0_tail + TAIL // 2)
    return i0, i1


def _bf16_to_f32(x_bf16):
    """Fast bf16 -> fp32 (bit shift; avoids slow ml_dtypes casting)."""
    u = x_bf16.view(np.uint16).astype(np.uint32) << 16
    return u.view(np.float32)


def _top16_rows(regs):
    """Exact top-16 positions per row of (B, HM) via two-level group maxes."""
    GH = 16                                          # host group size
    ng = regs.shape[1] // GH                         # 392 groups
    g = regs.reshape(B, ng, GH)
    gmax = g.max(axis=2)                             # (B, ng)
    gsel = np.argpartition(-gmax, TOPK - 1, axis=1)[:, :TOPK]   # (B, 16)
    cand = np.take_along_axis(g, gsel[:, :, None], axis=1)      # (B, 16, GH)
    cand = cand.reshape(B, TOPK * GH)
    fsel = np.argpartition(-cand, TOPK - 1, axis=1)[:, :TOPK]   # (B, 16)
    grp = np.take_along_axis(gsel, fsel // GH, axis=1)
    return grp * GH + fsel % GH                      # hm indices (B, 16)


def kernel(query, memory_feat, memory_evidence, model_evidence, top_k):
    top_k = int(top_k)
    assert top_k == TOPK

    query = np.asarray(query, dtype=np.float32)
    memory_feat = np.asarray(memory_feat, dtype=np.float32)
    memory_evidence = np.asarray(memory_evidence, dtype=np.float32)
    model_evidence = np.asarray(model_evidence, dtype=np.float32)

    nc = _get_program()
    qhat, mhat, qtl, mts = _prep_inputs(query, memory_feat)

    in_maps = [{"mt": mts[c], "qt": qtl} for c in range(NCORES)]
    res = bass_utils.run_bass_kernel_spmd(nc, in_maps, core_ids=list(range(NCORES)))
    _cache["last_results"] = res

    # host-side top-16 pairs per core, resolved to 2x16 candidate indices
    cand_idx = np.empty((B, NCORES * 2 * TOPK), dtype=np.int64)
    for c in range(NCORES):
        regs = _bf16_to_f32(res.results[c]["ov"])    # (B, HM) pair values
        h = _top16_rows(regs)                        # (B, 16) hm indices
        i0, i1 = _pair_indices(h)
        idx = np.concatenate([i0, i1], axis=1)       # (B, 32) slab positions
        valid = idx < NLOC_REAL
        gidx = c * NLOC_REAL + np.clip(idx, 0, NLOC_REAL - 1)
        gidx[~valid] = -1
        cand_idx[:, c * 2 * TOPK:(c + 1) * 2 * TOPK] = gidx

    # exact fp32 rescore of the 256 candidates per query
    safe_idx = np.clip(cand_idx, 0, N - 1)
    mh_c = mhat[safe_idx]                            # (B, 256, D)
    s = np.einsum("bd,bkd->bk", qhat, mh_c).astype(np.float32)
    s[cand_idx < 0] = -np.inf

    order = np.argsort(-s, axis=1, kind="stable")[:, :TOPK]
    topv = np.take_along_axis(s, order, axis=1)
    topi = np.take_along_axis(cand_idx, order, axis=1)

    return _fuse_host(topv, topi, memory_evidence, model_evidence)
